# revision 21
# baseline (speedup 1.0000x reference)
"""Distributed Trainium2 Bass kernel for AltAttention (cosine-sim attention with
alibi bias + key padding mask + out projection).

Sharding (8 cores): core c -> batch b = c//4, heads [4*(c%4) .. 4*(c%4)+3].

Key ideas vs the v0 baseline:
 - Masked keys are compacted away on the host: only the ~1024 unmasked keys
   per batch are projected/scored (padded to NKP=1152 = 9 k-tiles; padding
   keys get ealibi == 0 so they contribute exactly nothing).
 - exp(alibi) is precomputed on the host (bf16).  On device the softmax
   numerator is exp(s) * ealibi: the Scalar engine does nothing but Exp in
   the attention loop, and the alibi application is a bf16 SBUF x SBUF DVE
   multiply (2x packed mode) instead of an f32 PSUM add.
 - The two query halves of each head run interleaved so the PE always has
   independent matmul work while an exp/multiply round-trip is in flight
   (2 score tiles + 2 PV accumulators = exactly 8 PSUM banks).
 - Output ownership is token-chunked: core c produces tokens
   [256c, 256c+256) of BOTH batches, which makes every AllToAll slot index
   batch-independent -> no duplicated sends, no receiver-side selection.
   Division by the softmax denominator happens sender-side via a
   reciprocal + DRAM-bounce partition broadcast (all off the PE/ACT path).
 - TWO AllToAlls (one per head pair) instead of four: collectives have a
   large fixed re-arm cost; the first overlaps the second pair's compute.
 - Final projection accumulates even channel tiles during the last
   collective and odd tiles after it; bias via a ones-row matmul.
"""

import numpy as np
import ml_dtypes

import concourse.bass as bass
import concourse.mybir as mybir
import concourse.tile as tile
from concourse import bacc
from concourse.bass_utils import run_bass_kernel_spmd

BF = ml_dtypes.bfloat16
F32 = mybir.dt.float32
F32R = mybir.dt.float32r
BF16 = mybir.dt.bfloat16
AF = mybir.ActivationFunctionType
ALU = mybir.AluOpType

B, N, C, H = 2, 2048, 1024, 16
D = C // H
LOG_MAX = float(np.log(1.0 / 0.01))
N_CORES = 8
HPC = 4               # heads per core
NKP_DEFAULT = 1152    # padded compacted key count (9 tiles of 128)

TRACE = False
_NCS = {}


def _build(nkp):
    ktn = nkp // 128
    nc = bacc.Bacc("TRN2", target_bir_lowering=False, debug=False,
                   num_devices=N_CORES)

    xT_e = nc.dram_tensor("xT", [C, N], BF16, kind="ExternalInput")
    xTk_e = nc.dram_tensor("xTk", [C, nkp], BF16, kind="ExternalInput")
    wq_e = nc.dram_tensor("wq", [C, 256], BF16, kind="ExternalInput")
    wk_e = nc.dram_tensor("wk", [C, 256], BF16, kind="ExternalInput")
    wv_e = nc.dram_tensor("wv", [C, 256], BF16, kind="ExternalInput")
    ea_e = nc.dram_tensor("ea", [HPC, nkp, N], BF16, kind="ExternalInput")
    eqs_e = nc.dram_tensor("eqs", [128, 4], F32R, kind="ExternalInput")
    ek2_e = nc.dram_tensor("ek2", [128, 2], F32R, kind="ExternalInput")
    f2_e = nc.dram_tensor("f2", [2, 128], F32R, kind="ExternalInput")
    onesb_e = nc.dram_tensor("onesb", [1, 128], BF16, kind="ExternalInput")
    projw_e = nc.dram_tensor("projw", [C, C], BF16, kind="ExternalInput")
    projb_e = nc.dram_tensor("projb", [1, C], BF16, kind="ExternalInput")
    out_e = nc.dram_tensor("out", [512, C], F32, kind="ExternalOutput")

    with tile.TileContext(nc) as tc:
        with (
            tc.tile_pool(name="consts", bufs=1) as cpool,
            tc.tile_pool(name="qn", bufs=1) as qn_pool,
            tc.tile_pool(name="kn", bufs=1) as kn_pool,
            tc.tile_pool(name="vsb", bufs=1) as v_pool,
            tc.tile_pool(name="ea", bufs=2 * ktn - 3) as ea_pool,
            tc.tile_pool(name="wqk", bufs=1) as wqk_pool,
            tc.tile_pool(name="dram", bufs=1, space="DRAM") as dram,
        ):
            eqs = cpool.tile([128, 4], F32R)
            ek2 = cpool.tile([128, 2], F32R)
            f2 = cpool.tile([2, 128], F32R)
            onesb = cpool.tile([1, 128], BF16)
            projb = cpool.tile([1, C], BF16)

            qn_sb = [qn_pool.tile([128, N], BF16, name=f"qn{m}") for m in range(2)]
            kn_sb = [kn_pool.tile([128, nkp], BF16, name=f"kn{m}") for m in range(2)]
            v_sb = v_pool.tile([128, ktn, HPC * 65], BF16)

            wq = wqk_pool.tile([128, 8, 256], BF16)
            wk = wqk_pool.tile([128, 8, 256], BF16)

            ea_t = {}

            def ea_dma(h, kt):
                t = ea_pool.tile([128, N], BF16, tag="ea", name=f"ea{h}_{kt}")
                ea_t[(h, kt)] = t
                eng = (nc.sync, nc.scalar, nc.gpsimd)[kt % 3]
                eng.dma_start(t[:], ea_e.ap()[h, kt * 128:(kt + 1) * 128, :])

            # one collective per head pair: slot s rows 0:64 = even head,
            # 64:128 = odd head of the pair; [8, 128, 256] bf16
            a2a_in = [dram.tile([8, 128, 256], BF16, name=f"a2ai{p}") for p in range(2)]
            a2a_out = [dram.tile([8, 128, 256], BF16, name=f"a2ao{p}") for p in range(2)]
            drcp = [dram.tile([1, 1024], F32, name=f"drcp{i}") for i in range(4)]

            # =================== PHASE A: q/k/v projection + normalize ======
            with (
                tc.tile_pool(name="xw", bufs=1) as xw_pool,
                tc.tile_pool(name="qkT", bufs=2) as qkT_pool,
                tc.tile_pool(name="sq", bufs=1) as sq_pool,
                tc.tile_pool(name="sm", bufs=2) as sm_pool,
                tc.tile_pool(name="psA", bufs=6, space="PSUM") as psA,
                tc.tile_pool(name="psS", bufs=1, space="PSUM") as psS,
            ):
                for t, e in ((eqs, eqs_e), (ek2, ek2_e), (f2, f2_e),
                             (onesb, onesb_e), (projb, projb_e)):
                    nc.gpsimd.dma_start(t[:], e.ap())

                wv = xw_pool.tile([128, 8, 256], BF16)
                xT = xw_pool.tile([128, 8, N], BF16)
                xTk = xw_pool.tile([128, 8, nkp], BF16)
                # split the phase-A loads over all three DMA queues:
                # sync: wk + xTk (k-proj critical path); scalar: xT[0:6];
                # gpsimd: consts + wq + wv + xT[6:8]; then ealibi h0 3-way.
                for ct in range(8):
                    nc.sync.dma_start(wk[:, ct, :], wk_e.ap()[ct * 128:(ct + 1) * 128, :])
                for ct in range(8):
                    nc.gpsimd.dma_start(wq[:, ct, :], wq_e.ap()[ct * 128:(ct + 1) * 128, :])
                    nc.gpsimd.dma_start(wv[:, ct, :], wv_e.ap()[ct * 128:(ct + 1) * 128, :])
                for ct in range(4):
                    nc.sync.dma_start(xTk[:, ct, :], xTk_e.ap()[ct * 128:(ct + 1) * 128, :])
                for ct in range(3):
                    nc.scalar.dma_start(xT[:, ct, :], xT_e.ap()[ct * 128:(ct + 1) * 128, :])
                ea_dma(0, 0)   # sync
                ea_dma(0, 1)   # scalar
                ea_dma(0, 2)   # gpsimd
                for ct in range(4, 8):
                    nc.sync.dma_start(xTk[:, ct, :], xTk_e.ap()[ct * 128:(ct + 1) * 128, :])
                for ct in range(3, 6):
                    nc.scalar.dma_start(xT[:, ct, :], xT_e.ap()[ct * 128:(ct + 1) * 128, :])
                for ct in range(6, 8):
                    nc.gpsimd.dma_start(xT[:, ct, :], xT_e.ap()[ct * 128:(ct + 1) * 128, :])
                for kt in range(3, ktn):
                    ea_dma(0, kt)

                for h in range(HPC):
                    nc.vector.memset(v_sb[:, :, h * 65 + 64], 1.0)

                # ---- normalize chain (emitted after the projection matmuls
                # of the group so the PE stream never blocks on ACT/DVE) ----
                def chain(mt, pss, nchunk, cw, elhs, kind, dve_copy=False):
                    tot = nchunk * cw
                    qkT = qkT_pool.tile([128, tot], F32R, tag="qkT",
                                        name=f"{kind}T{mt}")
                    sq = sq_pool.tile([128, tot], F32R, tag="sq",
                                      name=f"{kind}sq{mt}")
                    for i in range(nchunk):
                        sl = slice(i * cw, (i + 1) * cw)
                        if dve_copy:
                            nc.vector.tensor_copy(qkT[:, sl].bitcast(F32),
                                                  pss[i][:, 0:cw])
                        else:
                            nc.scalar.activation(qkT[:, sl], pss[i][:, 0:cw], AF.Copy)
                        nc.vector.tensor_tensor(sq[:, sl], pss[i][:, 0:cw],
                                                qkT[:, sl].bitcast(F32), ALU.mult)
                    rnorm = sm_pool.tile([2, tot], F32R, tag="rn",
                                         name=f"{kind}rn{mt}")
                    for i in range(nchunk):
                        sl = slice(i * cw, (i + 1) * cw)
                        ssp = psS.tile([2, 1024], F32, tag="ssp",
                                       name=f"{kind}ss{mt}{i}")
                        nc.tensor.matmul(ssp[:, 0:cw], elhs, sq[:, sl],
                                         start=True, stop=True)
                        rr = sm_pool.tile([2, 1024], F32, tag="rr",
                                          name=f"{kind}rr{mt}{i}")
                        nc.vector.reciprocal_approx_fast(rr[:, 0:cw], ssp[:, 0:cw])
                        nc.scalar.activation(rnorm[:, sl], rr[:, 0:cw], AF.Sqrt)
                    dst = qn_sb[mt] if kind == "q" else kn_sb[mt]
                    for i in range(nchunk):
                        sl = slice(i * cw, (i + 1) * cw)
                        rep = psA.tile([128, 512], F32, tag="ps512",
                                       name=f"{kind}rep{mt}{i}")
                        nc.tensor.matmul(rep[:, 0:cw], f2[:], rnorm[:, sl],
                                         start=True, stop=True)
                        nc.vector.tensor_tensor(dst[:, sl],
                                                qkT[:, sl].bitcast(F32),
                                                rep[:, 0:cw], ALU.mult)

                def k_proj(mt):
                    pss = [psA.tile([128, 512], F32, tag="ps512",
                                    name=f"k{mt}{nt}") for nt in range(3)]
                    for ct in range(8):
                        for nt in range(3):
                            nc.tensor.matmul(
                                pss[nt][:, 0:384],
                                wk[:, ct, mt * 128:(mt + 1) * 128],
                                xTk[:, ct, nt * 384:(nt + 1) * 384],
                                start=(ct == 0), stop=(ct == 7))
                    chain(mt, pss, 3, 384, ek2[:], "k")

                def q_proj(mt):
                    pss = [psA.tile([128, 512], F32, tag="ps512",
                                    name=f"q{mt}{nt}") for nt in range(4)]
                    for ct in range(8):
                        for nt in range(4):
                            nc.tensor.matmul(
                                pss[nt][:],
                                wq[:, ct, mt * 128:(mt + 1) * 128],
                                xT[:, ct, nt * 512:(nt + 1) * 512],
                                start=(ct == 0), stop=(ct == 7))
                    chain(mt, pss, 4, 512, eqs[:, 2 * mt:2 * mt + 2], "q")

                def v_proj(tt):
                    vps = psA.tile([128, 512], F32, tag="ps512", name=f"v{tt}")
                    for ct in range(8):
                        nc.tensor.matmul(vps[:, 0:256],
                                         xTk[:, ct, tt * 128:(tt + 1) * 128],
                                         wv[:, ct, :], start=(ct == 0), stop=(ct == 7))
                    nc.scalar.activation(
                        v_sb[:, tt].rearrange("p (h d) -> p h d", h=4)[:, :, 0:64],
                        vps[:, 0:256].rearrange("p (h d) -> p h d", h=4), AF.Copy)

                # only head-pair-0 inputs + v before attention; k1/q1 are
                # emitted inside phase B (borrowing score PSUM tiles) so the
                # h0/h1 attention starts as early as possible.
                k_proj(0)
                q_proj(0)
                for tt in range(ktn):
                    v_proj(tt)

            # =================== PHASE B: attention =========================
            with (
                tc.tile_pool(name="qkT2", bufs=1) as qkT2_pool,
                tc.tile_pool(name="sm2", bufs=1) as sm2_pool,
                tc.tile_pool(name="lx", bufs=2) as lx_pool,
                tc.tile_pool(name="aw", bufs=1) as aw_pool,
                tc.tile_pool(name="p0", bufs=4) as p0_pool,
                tc.tile_pool(name="pp", bufs=4) as p_pool,
                tc.tile_pool(name="st", bufs=2) as st_pool,
                tc.tile_pool(name="psSC", bufs=2, space="PSUM") as psSC,
                tc.tile_pool(name="psOA", bufs=2, space="PSUM") as psOA,
            ):
                projw = aw_pool.tile([128, 8, C], BF16)
                a_nm = aw_pool.tile([128, 8, 512], BF16)

                def late_proj(kind, w_sb, xe, nchunk, cw, elhs):
                    xcs = []
                    for nt in range(nchunk):
                        xc = lx_pool.tile([128, 8, 512], BF16, tag="lx",
                                          name=f"{kind}x{nt}")
                        xcs.append(xc)
                        for ct in range(8):
                            eng = (nc.sync, nc.scalar, nc.gpsimd)[ct % 3]
                            eng.dma_start(
                                xc[:, ct, 0:cw],
                                xe.ap()[ct * 128:(ct + 1) * 128,
                                        nt * cw:(nt + 1) * cw])
                    pss = []
                    for nt in range(nchunk):
                        t = psSC.tile([128, 1024], F32, tag="sc",
                                      name=f"{kind}f{nt}")
                        pss.append(t)
                        for ct in range(8):
                            nc.tensor.matmul(
                                t[:, 0:cw], w_sb[:, ct, 128:256],
                                xcs[nt][:, ct, 0:cw],
                                start=(ct == 0), stop=(ct == 7))
                    # normalize chain (DVE copies; ACT only does the sqrt)
                    tot = nchunk * cw
                    qkT = qkT2_pool.tile([128, tot], F32R, tag="qkT2",
                                         name=f"{kind}T1")
                    sq = qkT2_pool.tile([128, tot], F32R, tag="sq2",
                                        name=f"{kind}sq1")
                    for i in range(nchunk):
                        sl = slice(i * cw, (i + 1) * cw)
                        nc.vector.tensor_copy(qkT[:, sl].bitcast(F32),
                                              pss[i][:, 0:cw])
                        nc.vector.tensor_tensor(sq[:, sl], pss[i][:, 0:cw],
                                                qkT[:, sl].bitcast(F32), ALU.mult)
                    rnorm = sm2_pool.tile([2, tot], F32R, tag="rn2",
                                          name=f"{kind}rn1")
                    for i in range(nchunk):
                        sl = slice(i * cw, (i + 1) * cw)
                        ssp = psOA.tile([65, 1024], F32, tag="oa",
                                        name=f"{kind}ss1{i}")
                        nc.tensor.matmul(ssp[0:2, 0:cw], elhs, sq[:, sl],
                                         start=True, stop=True)
                        rr = sm2_pool.tile([2, 1024], F32, tag="rr2",
                                           name=f"{kind}rr1{i}")
                        nc.vector.reciprocal_approx_fast(rr[:, 0:cw], ssp[0:2, 0:cw])
                        nc.scalar.activation(rnorm[:, sl], rr[:, 0:cw], AF.Sqrt)
                    dst = qn_sb[1] if kind == "q" else kn_sb[1]
                    for i in range(nchunk):
                        sl = slice(i * cw, (i + 1) * cw)
                        rep = psSC.tile([128, 1024], F32, tag="sc",
                                        name=f"{kind}rep1{i}")
                        nc.tensor.matmul(rep[:, 0:cw], f2[:], rnorm[:, sl],
                                         start=True, stop=True)
                        nc.vector.tensor_tensor(dst[:, sl],
                                                qkT[:, sl].bitcast(F32),
                                                rep[:, 0:cw], ALU.mult)

                for h in range(HPC):
                    mtq = h // 2
                    off = 64 * (h % 2)
                    oa = {}
                    sc_t = {}
                    p_t = {}
                    for qc in range(2):
                        oa[qc] = psOA.tile([65, 1024], F32, tag="oa",
                                           name=f"oa{h}{qc}")

                    def sc_mm(qc, kt):
                        sc = psSC.tile([128, 1024], F32, tag="sc",
                                       name=f"sc{h}{qc}{kt}")
                        sc_t[qc, kt] = sc
                        for j in range(2):
                            q0 = qc * 1024 + j * 512
                            nc.tensor.matmul(
                                sc[:, j * 512:(j + 1) * 512],
                                kn_sb[mtq][off:off + 64, kt * 128:(kt + 1) * 128],
                                qn_sb[mtq][off:off + 64, q0:q0 + 512],
                                start=True, stop=True)

                    def pv_mm(qc, kt):
                        for j in range(2):
                            nc.tensor.matmul(
                                oa[qc][:, j * 512:(j + 1) * 512],
                                v_sb[:, kt, h * 65:h * 65 + 65],
                                p_t[qc, kt][:, j * 512:(j + 1) * 512],
                                start=(kt == 0), stop=(kt == ktn - 1))

                    sc_mm(0, 0)
                    sc_mm(1, 0)
                    for kt in range(ktn):
                        for qc in range(2):
                            p0 = p0_pool.tile([128, 1024], BF16, tag="p0",
                                              name=f"p0_{h}{qc}{kt}")
                            nc.scalar.activation(p0[:], sc_t[qc, kt][:], AF.Exp)
                            p = p_pool.tile([128, 1024], BF16, tag="p",
                                            name=f"p{h}{qc}{kt}")
                            p_t[qc, kt] = p
                            nc.vector.tensor_tensor(
                                p[:], p0[:],
                                ea_t[(h, kt)][:, qc * 1024:(qc + 1) * 1024],
                                ALU.mult)
                        for qc in range(2):
                            if kt + 1 < ktn:
                                sc_mm(qc, kt + 1)
                            pv_mm(qc, kt)
                        if h + 1 < HPC:
                            ea_dma(h + 1, kt)
                        if h == 1 and kt < 8:
                            eng = nc.sync if kt % 2 == 0 else nc.scalar
                            eng.dma_start(projw[:, kt, :],
                                          projw_e.ap()[kt * 128:(kt + 1) * 128, :])

                    for qc in range(2):
                        # denominator: copy ones-row, reciprocal, broadcast via
                        # a DRAM bounce (stride-0 partition read), divide.
                        dd = (h % 2) * 2 + qc
                        dens = st_pool.tile([1, 1024], F32, tag="dens",
                                            name=f"dens{h}{qc}")
                        nc.vector.tensor_copy(dens[:], oa[qc][64:65, :])
                        num0 = st_pool.tile([64, 1024], BF16, tag="num0",
                                            name=f"num0{h}{qc}")
                        nc.vector.tensor_copy(num0[:], oa[qc][0:64, :])
                        rcp = st_pool.tile([1, 1024], F32, tag="rcp",
                                           name=f"rcp{h}{qc}")
                        nc.vector.reciprocal_approx_fast(rcp[:], dens[:])
                        nc.sync.dma_start(drcp[dd][:, :], rcp[:])
                        repb = st_pool.tile([64, 1024], F32, tag="repb",
                                            name=f"repb{h}{qc}")
                        nc.sync.dma_start(
                            repb[:], drcp[dd][:, :].partition_broadcast(64))
                        stg = st_pool.tile([64, 1024], BF16, tag="stg",
                                           name=f"stg{h}{qc}")
                        nc.vector.tensor_tensor(stg[:], num0[:],
                                                repb[:], ALU.mult)
                        nc.gpsimd.dma_start(
                            a2a_in[h // 2].rearrange(
                                "(q i) (hp p) t -> q hp p i t", q=2, hp=2)[qc, h % 2],
                            stg[:].rearrange("p (i t) -> p i t", i=4))

                    if h == 0:
                        late_proj("k", wk, xTk_e, 3, 384, ek2[:])
                    if h == 1:
                        late_proj("q", wq, xT_e, 4, 512, eqs[:, 2:4])

                    if h % 2 == 1:
                        par = h // 2
                        nc.gpsimd.collective_compute(
                            "AllToAll", ALU.bypass,
                            replica_groups=[list(range(N_CORES))],
                            ins=[a2a_in[par].opt()],
                            outs=[a2a_out[par].opt()],
                        )
                        dst = a_nm[:].rearrange("p j (b t) -> p j b t", b=2)
                        for bb in range(2):
                            nc.gpsimd.dma_start(
                                dst[:, par::2, bb, :],
                                a2a_out[par].rearrange(
                                    "(b g) p t -> b p g t", b=2)[bb])

            # =================== PHASE D: output projection =================
            with (
                tc.tile_pool(name="od", bufs=1) as od_pool,
                tc.tile_pool(name="psDO", bufs=1, space="PSUM") as psDO,
            ):
                ops = {}
                for tt in range(4):
                    for co in range(2):
                        ops[tt, co] = psDO.tile([128, 512], F32, tag=f"do{tt}{co}",
                                                name=f"do{tt}{co}")
                # even channel tiles overlap the last collective's latency
                for j in (0, 2, 4, 6, 1, 3, 5, 7):
                    for tt in range(4):
                        for co in range(2):
                            nc.tensor.matmul(ops[tt, co][:],
                                             a_nm[:, j, tt * 128:(tt + 1) * 128],
                                             projw[:, j, co * 512:(co + 1) * 512],
                                             start=(j == 0), stop=False)
                o_sb = od_pool.tile([128, 4, C], F32)
                for tt in range(4):
                    for co in range(2):
                        nc.tensor.matmul(ops[tt, co][:], onesb[:],
                                         projb[:, co * 512:(co + 1) * 512],
                                         start=False, stop=True)
                        if co == 0:
                            nc.scalar.activation(o_sb[:, tt, co * 512:(co + 1) * 512],
                                                 ops[tt, co][:], AF.Copy)
                        else:
                            nc.vector.tensor_copy(o_sb[:, tt, co * 512:(co + 1) * 512],
                                                  ops[tt, co][:])
                        oeng = (nc.sync, nc.scalar, nc.gpsimd)[(2 * tt + co) % 3]
                        oeng.dma_start(
                            out_e.ap()[tt * 128:(tt + 1) * 128,
                                       co * 512:(co + 1) * 512],
                            o_sb[:, tt, co * 512:(co + 1) * 512])

    nc.compile()
    return nc


def _get_nc(nkp):
    if nkp not in _NCS:
        _NCS[nkp] = _build(nkp)
    return _NCS[nkp]


def kernel(x, padding_mask, alibi_bias, qkv_w, proj_w, proj_b, logit_scale):
    x = np.asarray(x, np.float32)
    padding_mask = np.asarray(padding_mask, bool)
    alibi_bias = np.asarray(alibi_bias, np.float32)
    qkv_w = np.asarray(qkv_w, np.float32)
    proj_w = np.asarray(proj_w, np.float32)
    proj_b = np.asarray(proj_b, np.float32)
    lsc = np.minimum(np.asarray(logit_scale, np.float32).reshape(H), LOG_MAX)

    kept = [np.flatnonzero(~padding_mask[b]) for b in range(B)]
    nk_max = max(len(k) for k in kept)
    nkp = max(NKP_DEFAULT, -(-nk_max // 128) * 128)
    nc = _get_nc(nkp)

    f2 = np.zeros((2, 128), np.float32)
    f2[0, 0:64] = 1.0
    f2[1, 64:128] = 1.0
    ek2 = np.zeros((128, 2), np.float32)
    ek2[0:64, 0] = 1.0
    ek2[64:128, 1] = 1.0
    onesb = np.ones((1, 128), np.float32).astype(BF)
    projw = np.ascontiguousarray(proj_w.T).astype(BF)          # [c_in, c_out]
    projb = proj_b.reshape(1, C).astype(BF)

    in_maps = []
    for c in range(N_CORES):
        b = c // 4
        hs = [4 * (c % 4) + i for i in range(4)]
        idx = kept[b]
        idxp = np.concatenate([idx, np.zeros(nkp - len(idx), np.int64)])
        xT = np.ascontiguousarray(x[b].T).astype(BF)
        xTk = np.ascontiguousarray(x[b].T[:, idxp]).astype(BF)
        wq = np.ascontiguousarray(
            np.concatenate([qkv_w[h * D:(h + 1) * D, :] for h in hs], 0).T).astype(BF)
        wk = np.ascontiguousarray(
            np.concatenate([qkv_w[C + h * D:C + (h + 1) * D, :] for h in hs], 0).T).astype(BF)
        wv = np.ascontiguousarray(
            np.concatenate([qkv_w[2 * C + h * D:2 * C + (h + 1) * D, :] for h in hs], 0).T).astype(BF)
        # ealibi: [h, k_kept, q], exp(alibi); padded key rows -> 0
        ea = np.exp(alibi_bias[b][hs][:, :, idxp].transpose(0, 2, 1))
        if len(idx) < nkp:
            ea[:, len(idx):, :] = 0.0
        ea = np.ascontiguousarray(ea).astype(BF)
        eqs = np.zeros((128, 4), np.float32)
        for j in range(4):
            eqs[(j % 2) * 64:(j % 2) * 64 + 64, j] = np.exp(-2.0 * lsc[hs[j]])
        in_maps.append({
            "xT": xT, "xTk": xTk, "wq": wq, "wk": wk, "wv": wv, "ea": ea,
            "eqs": eqs, "ek2": ek2, "f2": f2,
            "onesb": onesb, "projw": projw, "projb": projb,
        })

    res = run_bass_kernel_spmd(nc, in_maps, core_ids=list(range(N_CORES)),
                               trace=TRACE)
    if TRACE:
        kernel.last_exec_time_ns = res.exec_time_ns
        kernel.last_results = res

    out = np.empty((B, N, C), np.float32)
    for c in range(N_CORES):
        out[0, 256 * c:256 * (c + 1), :] = res.results[c]["out"][0:256]
        out[1, 256 * c:256 * (c + 1), :] = res.results[c]["out"][256:512]
    return out


# revision 22
# speedup vs baseline: 1.0494x; 1.0494x over previous
"""Distributed Trainium2 Bass kernel for AltAttention (cosine-sim attention with
alibi bias + key padding mask + out projection).

Sharding (8 cores): core c -> batch b = c//4, heads [4*(c%4) .. 4*(c%4)+3].

Key ideas vs the v0 baseline:
 - Masked keys are compacted away on the host: only the ~1024 unmasked keys
   per batch are projected/scored (padded to NKP=1152 = 9 k-tiles; padding
   keys get ealibi == 0 so they contribute exactly nothing).
 - exp(alibi) is precomputed on the host (bf16).  On device the softmax
   numerator is exp(s) * ealibi: the Scalar engine does nothing but Exp in
   the attention loop, and the alibi application is a bf16 SBUF x SBUF DVE
   multiply (2x packed mode) instead of an f32 PSUM add.
 - The two query halves of each head run interleaved so the PE always has
   independent matmul work while an exp/multiply round-trip is in flight
   (2 score tiles + 2 PV accumulators = exactly 8 PSUM banks).
 - Output ownership is token-chunked: core c produces tokens
   [256c, 256c+256) of BOTH batches, which makes every AllToAll slot index
   batch-independent -> no duplicated sends, no receiver-side selection.
   Division by the softmax denominator happens sender-side via a
   reciprocal + DRAM-bounce partition broadcast (all off the PE/ACT path).
 - TWO AllToAlls (one per head pair) instead of four: collectives have a
   large fixed re-arm cost; the first overlaps the second pair's compute.
 - Final projection accumulates even channel tiles during the last
   collective and odd tiles after it; bias via a ones-row matmul.
"""

import numpy as np
import ml_dtypes

import concourse.bass as bass
import concourse.mybir as mybir
import concourse.tile as tile
from concourse import bacc
from concourse.bass_utils import run_bass_kernel_spmd

BF = ml_dtypes.bfloat16
F32 = mybir.dt.float32
F32R = mybir.dt.float32r
BF16 = mybir.dt.bfloat16
AF = mybir.ActivationFunctionType
ALU = mybir.AluOpType

B, N, C, H = 2, 2048, 1024, 16
D = C // H
LOG_MAX = float(np.log(1.0 / 0.01))
N_CORES = 8
HPC = 4               # heads per core
NKP_DEFAULT = 1152    # padded compacted key count (9 tiles of 128)

TRACE = False
_NCS = {}


def _build(nkp):
    ktn = nkp // 128
    nc = bacc.Bacc("TRN2", target_bir_lowering=False, debug=False,
                   num_devices=N_CORES)

    xT_e = nc.dram_tensor("xT", [C, N], BF16, kind="ExternalInput")
    xTk_e = nc.dram_tensor("xTk", [C, nkp], BF16, kind="ExternalInput")
    wq_e = nc.dram_tensor("wq", [C, 256], BF16, kind="ExternalInput")
    wk_e = nc.dram_tensor("wk", [C, 256], BF16, kind="ExternalInput")
    wv_e = nc.dram_tensor("wv", [C, 256], BF16, kind="ExternalInput")
    ea_e = nc.dram_tensor("ea", [HPC, nkp, N], BF16, kind="ExternalInput")
    eqs_e = nc.dram_tensor("eqs", [128, 4], F32R, kind="ExternalInput")
    ek2_e = nc.dram_tensor("ek2", [128, 2], F32R, kind="ExternalInput")
    f2_e = nc.dram_tensor("f2", [2, 128], F32R, kind="ExternalInput")
    onesb_e = nc.dram_tensor("onesb", [1, 128], BF16, kind="ExternalInput")
    projw_e = nc.dram_tensor("projw", [C, C], BF16, kind="ExternalInput")
    projb_e = nc.dram_tensor("projb", [1, C], BF16, kind="ExternalInput")
    out_e = nc.dram_tensor("out", [512, C], F32, kind="ExternalOutput")

    with tile.TileContext(nc) as tc:
        with (
            tc.tile_pool(name="consts", bufs=1) as cpool,
            tc.tile_pool(name="qn", bufs=1) as qn_pool,
            tc.tile_pool(name="kn", bufs=1) as kn_pool,
            tc.tile_pool(name="vsb", bufs=1) as v_pool,
            tc.tile_pool(name="ea", bufs=2 * ktn - 1) as ea_pool,
            tc.tile_pool(name="dram", bufs=1, space="DRAM") as dram,
        ):
            eqs = cpool.tile([128, 4], F32R)
            ek2 = cpool.tile([128, 2], F32R)
            f2 = cpool.tile([2, 128], F32R)
            onesb = cpool.tile([1, 128], BF16)
            projb = cpool.tile([1, C], BF16)

            qn_sb = [qn_pool.tile([128, N], BF16, name=f"qn{m}") for m in range(2)]
            kn_sb = [kn_pool.tile([128, nkp], BF16, name=f"kn{m}") for m in range(2)]
            v_sb = v_pool.tile([128, ktn, HPC * 65], BF16)

            ea_t = {}

            def ea_dma(h, kt):
                t = ea_pool.tile([128, N], BF16, tag="ea", name=f"ea{h}_{kt}")
                ea_t[(h, kt)] = t
                eng = (nc.sync, nc.scalar, nc.gpsimd)[kt % 3]
                eng.dma_start(t[:], ea_e.ap()[h, kt * 128:(kt + 1) * 128, :])

            # one collective per head pair: slot s rows 0:64 = even head,
            # 64:128 = odd head of the pair; [8, 128, 256] bf16
            a2a_in = [dram.tile([8, 128, 256], BF16, name=f"a2ai{p}") for p in range(2)]
            a2a_out = [dram.tile([8, 128, 256], BF16, name=f"a2ao{p}") for p in range(2)]
            drcp = [dram.tile([1, 1024], F32, name=f"drcp{i}") for i in range(4)]

            # =================== PHASE A: q/k/v projection + normalize ======
            with (
                tc.tile_pool(name="xw", bufs=1) as xw_pool,
                tc.tile_pool(name="qkT", bufs=2) as qkT_pool,
                tc.tile_pool(name="sq", bufs=1) as sq_pool,
                tc.tile_pool(name="sm", bufs=2) as sm_pool,
                tc.tile_pool(name="psA", bufs=6, space="PSUM") as psA,
                tc.tile_pool(name="psS", bufs=1, space="PSUM") as psS,
            ):
                for t, e in ((eqs, eqs_e), (ek2, ek2_e), (f2, f2_e),
                             (onesb, onesb_e), (projb, projb_e)):
                    nc.gpsimd.dma_start(t[:], e.ap())

                wq = xw_pool.tile([128, 8, 256], BF16)
                wk = xw_pool.tile([128, 8, 256], BF16)
                wv = xw_pool.tile([128, 8, 256], BF16)
                xT = xw_pool.tile([128, 8, N], BF16)
                xTk = xw_pool.tile([128, 8, nkp], BF16)
                # split the phase-A loads over all three DMA queues:
                # sync: wk + xTk (k-proj critical path); scalar: xT[0:6];
                # gpsimd: consts + wq + wv + xT[6:8]; then ealibi h0 3-way.
                for ct in range(8):
                    nc.sync.dma_start(wk[:, ct, :], wk_e.ap()[ct * 128:(ct + 1) * 128, :])
                for ct in range(8):
                    nc.gpsimd.dma_start(wq[:, ct, :], wq_e.ap()[ct * 128:(ct + 1) * 128, :])
                    nc.gpsimd.dma_start(wv[:, ct, :], wv_e.ap()[ct * 128:(ct + 1) * 128, :])
                for ct in range(4):
                    nc.sync.dma_start(xTk[:, ct, :], xTk_e.ap()[ct * 128:(ct + 1) * 128, :])
                for ct in range(3):
                    nc.scalar.dma_start(xT[:, ct, :], xT_e.ap()[ct * 128:(ct + 1) * 128, :])
                ea_dma(0, 0)   # sync
                ea_dma(0, 1)   # scalar
                ea_dma(0, 2)   # gpsimd
                for ct in range(4, 8):
                    nc.sync.dma_start(xTk[:, ct, :], xTk_e.ap()[ct * 128:(ct + 1) * 128, :])
                for ct in range(3, 6):
                    nc.scalar.dma_start(xT[:, ct, :], xT_e.ap()[ct * 128:(ct + 1) * 128, :])
                for ct in range(6, 8):
                    nc.gpsimd.dma_start(xT[:, ct, :], xT_e.ap()[ct * 128:(ct + 1) * 128, :])
                for kt in range(3, ktn):
                    ea_dma(0, kt)

                for h in range(HPC):
                    nc.vector.memset(v_sb[:, :, h * 65 + 64], 1.0)

                # ---- normalize chain (emitted after the projection matmuls
                # of the group so the PE stream never blocks on ACT/DVE) ----
                def chain(mt, pss, nchunk, cw, elhs, kind, dve_copy=False):
                    tot = nchunk * cw
                    qkT = qkT_pool.tile([128, tot], F32R, tag="qkT",
                                        name=f"{kind}T{mt}")
                    sq = sq_pool.tile([128, tot], F32R, tag="sq",
                                      name=f"{kind}sq{mt}")
                    for i in range(nchunk):
                        sl = slice(i * cw, (i + 1) * cw)
                        if dve_copy:
                            nc.vector.tensor_copy(qkT[:, sl].bitcast(F32),
                                                  pss[i][:, 0:cw])
                        else:
                            nc.scalar.activation(qkT[:, sl], pss[i][:, 0:cw], AF.Copy)
                        nc.vector.tensor_tensor(sq[:, sl], pss[i][:, 0:cw],
                                                qkT[:, sl].bitcast(F32), ALU.mult)
                    rnorm = sm_pool.tile([2, tot], F32R, tag="rn",
                                         name=f"{kind}rn{mt}")
                    for i in range(nchunk):
                        sl = slice(i * cw, (i + 1) * cw)
                        ssp = psS.tile([2, 1024], F32, tag="ssp",
                                       name=f"{kind}ss{mt}{i}")
                        nc.tensor.matmul(ssp[:, 0:cw], elhs, sq[:, sl],
                                         start=True, stop=True)
                        rr = sm_pool.tile([2, 1024], F32, tag="rr",
                                          name=f"{kind}rr{mt}{i}")
                        nc.vector.reciprocal_approx_fast(rr[:, 0:cw], ssp[:, 0:cw])
                        nc.scalar.activation(rnorm[:, sl], rr[:, 0:cw], AF.Sqrt)
                    dst = qn_sb[mt] if kind == "q" else kn_sb[mt]
                    for i in range(nchunk):
                        sl = slice(i * cw, (i + 1) * cw)
                        rep = psA.tile([128, 512], F32, tag="ps512",
                                       name=f"{kind}rep{mt}{i}")
                        nc.tensor.matmul(rep[:, 0:cw], f2[:], rnorm[:, sl],
                                         start=True, stop=True)
                        nc.vector.tensor_tensor(dst[:, sl],
                                                qkT[:, sl].bitcast(F32),
                                                rep[:, 0:cw], ALU.mult)

                def k_proj(mt):
                    pss = [psA.tile([128, 512], F32, tag="ps512",
                                    name=f"k{mt}{nt}") for nt in range(3)]
                    for ct in range(8):
                        for nt in range(3):
                            nc.tensor.matmul(
                                pss[nt][:, 0:384],
                                wk[:, ct, mt * 128:(mt + 1) * 128],
                                xTk[:, ct, nt * 384:(nt + 1) * 384],
                                start=(ct == 0), stop=(ct == 7))
                    chain(mt, pss, 3, 384, ek2[:], "k")

                def q_proj(mt):
                    pss = [psA.tile([128, 512], F32, tag="ps512",
                                    name=f"q{mt}{nt}") for nt in range(4)]
                    for ct in range(8):
                        for nt in range(4):
                            nc.tensor.matmul(
                                pss[nt][:],
                                wq[:, ct, mt * 128:(mt + 1) * 128],
                                xT[:, ct, nt * 512:(nt + 1) * 512],
                                start=(ct == 0), stop=(ct == 7))
                    chain(mt, pss, 4, 512, eqs[:, 2 * mt:2 * mt + 2], "q")

                def v_proj(tt):
                    vps = psA.tile([128, 512], F32, tag="ps512", name=f"v{tt}")
                    for ct in range(8):
                        nc.tensor.matmul(vps[:, 0:256],
                                         xTk[:, ct, tt * 128:(tt + 1) * 128],
                                         wv[:, ct, :], start=(ct == 0), stop=(ct == 7))
                    nc.scalar.activation(
                        v_sb[:, tt].rearrange("p (h d) -> p h d", h=4)[:, :, 0:64],
                        vps[:, 0:256].rearrange("p (h d) -> p h d", h=4), AF.Copy)

                # head-0 inputs first (k0, q0), then v, then the second head
                # pair; chains are emitted after each projection pair so the
                # PE stream is never blocked by a DVE/ACT round trip.
                k_proj(0)
                q_proj(0)
                for tt in range(ktn):
                    v_proj(tt)
                k_proj(1)
                q_proj(1)

            # =================== PHASE B: attention =========================
            with (
                tc.tile_pool(name="aw", bufs=1) as aw_pool,
                tc.tile_pool(name="p0", bufs=4) as p0_pool,
                tc.tile_pool(name="pp", bufs=4) as p_pool,
                tc.tile_pool(name="st", bufs=3) as st_pool,
                tc.tile_pool(name="psSC", bufs=2, space="PSUM") as psSC,
                tc.tile_pool(name="psOA", bufs=2, space="PSUM") as psOA,
            ):
                projw = aw_pool.tile([128, 8, C], BF16)
                a_nm = aw_pool.tile([128, 8, 512], BF16)

                for h in range(HPC):
                    mtq = h // 2
                    off = 64 * (h % 2)
                    oa = {}
                    sc_t = {}
                    p_t = {}
                    for qc in range(2):
                        oa[qc] = psOA.tile([65, 1024], F32, tag="oa",
                                           name=f"oa{h}{qc}")

                    def sc_mm(qc, kt):
                        sc = psSC.tile([128, 1024], F32, tag="sc",
                                       name=f"sc{h}{qc}{kt}")
                        sc_t[qc, kt] = sc
                        for j in range(2):
                            q0 = qc * 1024 + j * 512
                            nc.tensor.matmul(
                                sc[:, j * 512:(j + 1) * 512],
                                kn_sb[mtq][off:off + 64, kt * 128:(kt + 1) * 128],
                                qn_sb[mtq][off:off + 64, q0:q0 + 512],
                                start=True, stop=True)

                    def pv_mm(qc, kt):
                        for j in range(2):
                            nc.tensor.matmul(
                                oa[qc][:, j * 512:(j + 1) * 512],
                                v_sb[:, kt, h * 65:h * 65 + 65],
                                p_t[qc, kt][:, j * 512:(j + 1) * 512],
                                start=(kt == 0), stop=(kt == ktn - 1))

                    sc_mm(0, 0)
                    sc_mm(1, 0)
                    for kt in range(ktn):
                        for qc in range(2):
                            p0 = p0_pool.tile([128, 1024], BF16, tag="p0",
                                              name=f"p0_{h}{qc}{kt}")
                            nc.scalar.activation(p0[:], sc_t[qc, kt][:], AF.Exp)
                            p = p_pool.tile([128, 1024], BF16, tag="p",
                                            name=f"p{h}{qc}{kt}")
                            p_t[qc, kt] = p
                            nc.vector.tensor_tensor(
                                p[:], p0[:],
                                ea_t[(h, kt)][:, qc * 1024:(qc + 1) * 1024],
                                ALU.mult)
                        for qc in range(2):
                            if kt + 1 < ktn:
                                sc_mm(qc, kt + 1)
                            pv_mm(qc, kt)
                        if h + 1 < HPC:
                            ea_dma(h + 1, kt)
                        if h == 1 and kt < 8:
                            eng = nc.sync if kt % 2 == 0 else nc.scalar
                            eng.dma_start(projw[:, kt, :],
                                          projw_e.ap()[kt * 128:(kt + 1) * 128, :])

                    for qc in range(2):
                        # denominator: copy ones-row, reciprocal, broadcast via
                        # a DRAM bounce (stride-0 partition read), divide.
                        dd = (h % 2) * 2 + qc
                        dens = st_pool.tile([1, 1024], F32, tag="dens",
                                            name=f"dens{h}{qc}")
                        nc.vector.tensor_copy(dens[:], oa[qc][64:65, :])
                        num0 = st_pool.tile([64, 1024], BF16, tag="num0",
                                            name=f"num0{h}{qc}")
                        nc.vector.tensor_copy(num0[:], oa[qc][0:64, :])
                        rcp = st_pool.tile([1, 1024], F32, tag="rcp",
                                           name=f"rcp{h}{qc}")
                        nc.vector.reciprocal_approx_fast(rcp[:], dens[:])
                        nc.sync.dma_start(drcp[dd][:, :], rcp[:])
                        repb = st_pool.tile([64, 1024], F32, tag="repb",
                                            name=f"repb{h}{qc}")
                        nc.sync.dma_start(
                            repb[:], drcp[dd][:, :].partition_broadcast(64))
                        stg = st_pool.tile([64, 1024], BF16, tag="stg",
                                           name=f"stg{h}{qc}")
                        nc.vector.tensor_tensor(stg[:], num0[:],
                                                repb[:], ALU.mult)
                        nc.gpsimd.dma_start(
                            a2a_in[h // 2].rearrange(
                                "(q i) (hp p) t -> q hp p i t", q=2, hp=2)[qc, h % 2],
                            stg[:].rearrange("p (i t) -> p i t", i=4))

                    if h % 2 == 1:
                        par = h // 2
                        nc.gpsimd.collective_compute(
                            "AllToAll", ALU.bypass,
                            replica_groups=[list(range(N_CORES))],
                            ins=[a2a_in[par].opt()],
                            outs=[a2a_out[par].opt()],
                        )
                        dst = a_nm[:].rearrange("p j (b t) -> p j b t", b=2)
                        for bb in range(2):
                            nc.gpsimd.dma_start(
                                dst[:, par::2, bb, :],
                                a2a_out[par].rearrange(
                                    "(b g) p t -> b p g t", b=2)[bb])

            # =================== PHASE D: output projection =================
            with (
                tc.tile_pool(name="od", bufs=1) as od_pool,
                tc.tile_pool(name="psDO", bufs=1, space="PSUM") as psDO,
            ):
                ops = {}
                for tt in range(4):
                    for co in range(2):
                        ops[tt, co] = psDO.tile([128, 512], F32, tag=f"do{tt}{co}",
                                                name=f"do{tt}{co}")
                # even channel tiles overlap the last collective's latency
                for j in (0, 2, 4, 6, 1, 3, 5, 7):
                    for tt in range(4):
                        for co in range(2):
                            nc.tensor.matmul(ops[tt, co][:],
                                             a_nm[:, j, tt * 128:(tt + 1) * 128],
                                             projw[:, j, co * 512:(co + 1) * 512],
                                             start=(j == 0), stop=False)
                o_sb = od_pool.tile([128, 4, C], F32)
                for tt in range(4):
                    for co in range(2):
                        nc.tensor.matmul(ops[tt, co][:], onesb[:],
                                         projb[:, co * 512:(co + 1) * 512],
                                         start=False, stop=True)
                        if co == 0:
                            nc.scalar.activation(o_sb[:, tt, co * 512:(co + 1) * 512],
                                                 ops[tt, co][:], AF.Copy)
                        else:
                            nc.vector.tensor_copy(o_sb[:, tt, co * 512:(co + 1) * 512],
                                                  ops[tt, co][:])
                        oeng = (nc.sync, nc.scalar, nc.gpsimd)[(2 * tt + co) % 3]
                        oeng.dma_start(
                            out_e.ap()[tt * 128:(tt + 1) * 128,
                                       co * 512:(co + 1) * 512],
                            o_sb[:, tt, co * 512:(co + 1) * 512])

    nc.compile()
    return nc


def _get_nc(nkp):
    if nkp not in _NCS:
        _NCS[nkp] = _build(nkp)
    return _NCS[nkp]


def kernel(x, padding_mask, alibi_bias, qkv_w, proj_w, proj_b, logit_scale):
    x = np.asarray(x, np.float32)
    padding_mask = np.asarray(padding_mask, bool)
    alibi_bias = np.asarray(alibi_bias, np.float32)
    qkv_w = np.asarray(qkv_w, np.float32)
    proj_w = np.asarray(proj_w, np.float32)
    proj_b = np.asarray(proj_b, np.float32)
    lsc = np.minimum(np.asarray(logit_scale, np.float32).reshape(H), LOG_MAX)

    kept = [np.flatnonzero(~padding_mask[b]) for b in range(B)]
    nk_max = max(len(k) for k in kept)
    nkp = max(NKP_DEFAULT, -(-nk_max // 128) * 128)
    nc = _get_nc(nkp)

    f2 = np.zeros((2, 128), np.float32)
    f2[0, 0:64] = 1.0
    f2[1, 64:128] = 1.0
    ek2 = np.zeros((128, 2), np.float32)
    ek2[0:64, 0] = 1.0
    ek2[64:128, 1] = 1.0
    onesb = np.ones((1, 128), np.float32).astype(BF)
    projw = np.ascontiguousarray(proj_w.T).astype(BF)          # [c_in, c_out]
    projb = proj_b.reshape(1, C).astype(BF)

    in_maps = []
    for c in range(N_CORES):
        b = c // 4
        hs = [4 * (c % 4) + i for i in range(4)]
        idx = kept[b]
        idxp = np.concatenate([idx, np.zeros(nkp - len(idx), np.int64)])
        xT = np.ascontiguousarray(x[b].T).astype(BF)
        xTk = np.ascontiguousarray(x[b].T[:, idxp]).astype(BF)
        wq = np.ascontiguousarray(
            np.concatenate([qkv_w[h * D:(h + 1) * D, :] for h in hs], 0).T).astype(BF)
        wk = np.ascontiguousarray(
            np.concatenate([qkv_w[C + h * D:C + (h + 1) * D, :] for h in hs], 0).T).astype(BF)
        wv = np.ascontiguousarray(
            np.concatenate([qkv_w[2 * C + h * D:2 * C + (h + 1) * D, :] for h in hs], 0).T).astype(BF)
        # ealibi: [h, k_kept, q], exp(alibi); padded key rows -> 0
        ea = np.exp(alibi_bias[b][hs][:, :, idxp].transpose(0, 2, 1))
        if len(idx) < nkp:
            ea[:, len(idx):, :] = 0.0
        ea = np.ascontiguousarray(ea).astype(BF)
        eqs = np.zeros((128, 4), np.float32)
        for j in range(4):
            eqs[(j % 2) * 64:(j % 2) * 64 + 64, j] = np.exp(-2.0 * lsc[hs[j]])
        in_maps.append({
            "xT": xT, "xTk": xTk, "wq": wq, "wk": wk, "wv": wv, "ea": ea,
            "eqs": eqs, "ek2": ek2, "f2": f2,
            "onesb": onesb, "projw": projw, "projb": projb,
        })

    res = run_bass_kernel_spmd(nc, in_maps, core_ids=list(range(N_CORES)),
                               trace=TRACE)
    if TRACE:
        kernel.last_exec_time_ns = res.exec_time_ns
        kernel.last_results = res

    out = np.empty((B, N, C), np.float32)
    for c in range(N_CORES):
        out[0, 256 * c:256 * (c + 1), :] = res.results[c]["out"][0:256]
        out[1, 256 * c:256 * (c + 1), :] = res.results[c]["out"][256:512]
    return out


# revision 23
# speedup vs baseline: 1.1677x; 1.1128x over previous
"""Distributed Trainium2 Bass kernel for AltAttention (cosine-sim attention with
alibi bias + key padding mask + out projection).

Sharding (8 cores): core c -> batch b = c//4, heads [4*(c%4) .. 4*(c%4)+3].

Key ideas vs the v0 baseline:
 - Masked keys are compacted away on the host: only the ~1024 unmasked keys
   per batch are projected/scored (padded to NKP=1152 = 9 k-tiles; padding
   keys get ealibi == 0 so they contribute exactly nothing).
 - exp(alibi) is precomputed on the host (bf16).  On device the softmax
   numerator is exp(s) * ealibi: the Scalar engine does nothing but Exp in
   the attention loop, and the alibi application is a bf16 SBUF x SBUF DVE
   multiply (2x packed mode) instead of an f32 PSUM add.
 - The two query halves of each head run interleaved so the PE always has
   independent matmul work while an exp/multiply round-trip is in flight
   (2 score tiles + 2 PV accumulators = exactly 8 PSUM banks).
 - Output ownership is token-chunked: core c produces tokens
   [256c, 256c+256) of BOTH batches, which makes every AllToAll slot index
   batch-independent -> no duplicated sends, no receiver-side selection.
   Division by the softmax denominator happens sender-side via a
   reciprocal + DRAM-bounce partition broadcast (all off the PE/ACT path).
 - TWO AllToAlls (one per head pair) instead of four: collectives have a
   large fixed re-arm cost; the first overlaps the second pair's compute.
 - Final projection accumulates even channel tiles during the last
   collective and odd tiles after it; bias via a ones-row matmul.
"""

import numpy as np
import ml_dtypes

import concourse.bass as bass
import concourse.mybir as mybir
import concourse.tile as tile
from concourse import bacc
from concourse.bass_utils import run_bass_kernel_spmd

BF = ml_dtypes.bfloat16
F32 = mybir.dt.float32
F32R = mybir.dt.float32r
BF16 = mybir.dt.bfloat16
AF = mybir.ActivationFunctionType
ALU = mybir.AluOpType

B, N, C, H = 2, 2048, 1024, 16
D = C // H
LOG_MAX = float(np.log(1.0 / 0.01))
N_CORES = 8
HPC = 4               # heads per core
NKP_DEFAULT = 1152    # padded compacted key count (9 tiles of 128)

TRACE = False
_NCS = {}


def _build(nkp):
    ktn = nkp // 128
    nc = bacc.Bacc("TRN2", target_bir_lowering=False, debug=False,
                   num_devices=N_CORES)

    xT_e = nc.dram_tensor("xT", [C, N], BF16, kind="ExternalInput")
    xTk_e = nc.dram_tensor("xTk", [C, nkp], BF16, kind="ExternalInput")
    wq_e = nc.dram_tensor("wq", [C, 256], BF16, kind="ExternalInput")
    wk_e = nc.dram_tensor("wk", [C, 256], BF16, kind="ExternalInput")
    wv_e = nc.dram_tensor("wv", [C, 256], BF16, kind="ExternalInput")
    ea_e = nc.dram_tensor("ea", [HPC, nkp, N], BF16, kind="ExternalInput")
    eqs_e = nc.dram_tensor("eqs", [128, 4], F32R, kind="ExternalInput")
    ek2_e = nc.dram_tensor("ek2", [128, 2], F32R, kind="ExternalInput")
    f2_e = nc.dram_tensor("f2", [2, 128], F32R, kind="ExternalInput")
    onesb_e = nc.dram_tensor("onesb", [1, 128], BF16, kind="ExternalInput")
    projw_e = nc.dram_tensor("projw", [C, C], BF16, kind="ExternalInput")
    projb_e = nc.dram_tensor("projb", [1, C], BF16, kind="ExternalInput")
    out_e = nc.dram_tensor("out", [512, C], F32, kind="ExternalOutput")

    with tile.TileContext(nc) as tc:
        with (
            tc.tile_pool(name="consts", bufs=1) as cpool,
            tc.tile_pool(name="qn", bufs=1) as qn_pool,
            tc.tile_pool(name="kn", bufs=1) as kn_pool,
            tc.tile_pool(name="vsb", bufs=1) as v_pool,
            tc.tile_pool(name="ea", bufs=2 * ktn - 1) as ea_pool,
            tc.tile_pool(name="dram", bufs=1, space="DRAM") as dram,
        ):
            eqs = cpool.tile([128, 4], F32R)
            ek2 = cpool.tile([128, 2], F32R)
            f2 = cpool.tile([2, 128], F32R)
            onesb = cpool.tile([1, 128], BF16)
            projb = cpool.tile([1, C], BF16)

            qn_sb = [qn_pool.tile([128, N], BF16, name=f"qn{m}") for m in range(2)]
            kn_sb = [kn_pool.tile([128, nkp], BF16, name=f"kn{m}") for m in range(2)]
            v_sb = v_pool.tile([128, ktn, HPC * 65], BF16)

            ea_t = {}

            def ea_dma(h, kt):
                t = ea_pool.tile([128, N], BF16, tag="ea", name=f"ea{h}_{kt}")
                ea_t[(h, kt)] = t
                eng = (nc.sync, nc.gpsimd)[kt % 2]
                eng.dma_start(t[:], ea_e.ap()[h, kt * 128:(kt + 1) * 128, :])

            # one collective per head pair: slot s rows 0:64 = even head,
            # 64:128 = odd head of the pair; [8, 128, 256] bf16
            a2a_in = [dram.tile([8, 128, 256], BF16, name=f"a2ai{p}") for p in range(2)]
            a2a_out = [dram.tile([8, 128, 256], BF16, name=f"a2ao{p}") for p in range(2)]
            drcp = [dram.tile([1, 1024], F32, name=f"drcp{i}") for i in range(4)]

            # =================== PHASE A: q/k/v projection + normalize ======
            with (
                tc.tile_pool(name="xw", bufs=1) as xw_pool,
                tc.tile_pool(name="qkT", bufs=2) as qkT_pool,
                tc.tile_pool(name="sq", bufs=1) as sq_pool,
                tc.tile_pool(name="sm", bufs=2) as sm_pool,
                tc.tile_pool(name="psA", bufs=6, space="PSUM") as psA,
                tc.tile_pool(name="psS", bufs=1, space="PSUM") as psS,
            ):
                for t, e in ((eqs, eqs_e), (ek2, ek2_e), (f2, f2_e),
                             (onesb, onesb_e), (projb, projb_e)):
                    nc.gpsimd.dma_start(t[:], e.ap())

                wq = xw_pool.tile([128, 8, 256], BF16)
                wk = xw_pool.tile([128, 8, 256], BF16)
                wv = xw_pool.tile([128, 8, 256], BF16)
                xT = xw_pool.tile([128, 8, N], BF16)
                xTk = xw_pool.tile([128, 8, nkp], BF16)
                # split the phase-A loads over all three DMA queues:
                # sync: wk + xTk (k-proj critical path); scalar: xT[0:6];
                # gpsimd: consts + wq + wv + xT[6:8]; then ealibi h0 3-way.
                for ct in range(8):
                    nc.sync.dma_start(wk[:, ct, :], wk_e.ap()[ct * 128:(ct + 1) * 128, :])
                for ct in range(8):
                    nc.gpsimd.dma_start(wq[:, ct, :], wq_e.ap()[ct * 128:(ct + 1) * 128, :])
                for ct in range(8):
                    nc.sync.dma_start(xTk[:, ct, :], xTk_e.ap()[ct * 128:(ct + 1) * 128, :])
                for ct in range(5):
                    nc.scalar.dma_start(xT[:, ct, :], xT_e.ap()[ct * 128:(ct + 1) * 128, :])
                for ct in range(8):
                    nc.gpsimd.dma_start(wv[:, ct, :], wv_e.ap()[ct * 128:(ct + 1) * 128, :])
                for ct in range(5, 8):
                    nc.gpsimd.dma_start(xT[:, ct, :], xT_e.ap()[ct * 128:(ct + 1) * 128, :])
                for kt in range(ktn):
                    ea_dma(0, kt)

                for h in range(HPC):
                    nc.vector.memset(v_sb[:, :, h * 65 + 64], 1.0)

                # ---- normalize chain (emitted after the projection matmuls
                # of the group so the PE stream never blocks on ACT/DVE) ----
                def chain(mt, pss, nchunk, cw, elhs, kind, dve_copy=False):
                    tot = nchunk * cw
                    qkT = qkT_pool.tile([128, tot], F32R, tag="qkT",
                                        name=f"{kind}T{mt}")
                    sq = sq_pool.tile([128, tot], F32R, tag="sq",
                                      name=f"{kind}sq{mt}")
                    for i in range(nchunk):
                        sl = slice(i * cw, (i + 1) * cw)
                        if dve_copy:
                            nc.vector.tensor_copy(qkT[:, sl].bitcast(F32),
                                                  pss[i][:, 0:cw])
                        else:
                            nc.scalar.activation(qkT[:, sl], pss[i][:, 0:cw], AF.Copy)
                        nc.vector.tensor_tensor(sq[:, sl], pss[i][:, 0:cw],
                                                qkT[:, sl].bitcast(F32), ALU.mult)
                    rnorm = sm_pool.tile([2, tot], F32R, tag="rn",
                                         name=f"{kind}rn{mt}")
                    for i in range(nchunk):
                        sl = slice(i * cw, (i + 1) * cw)
                        ssp = psS.tile([2, 1024], F32, tag="ssp",
                                       name=f"{kind}ss{mt}{i}")
                        nc.tensor.matmul(ssp[:, 0:cw], elhs, sq[:, sl],
                                         start=True, stop=True)
                        rr = sm_pool.tile([2, 1024], F32, tag="rr",
                                          name=f"{kind}rr{mt}{i}")
                        nc.vector.reciprocal_approx_fast(rr[:, 0:cw], ssp[:, 0:cw])
                        nc.scalar.activation(rnorm[:, sl], rr[:, 0:cw], AF.Sqrt)
                    dst = qn_sb[mt] if kind == "q" else kn_sb[mt]
                    for i in range(nchunk):
                        sl = slice(i * cw, (i + 1) * cw)
                        rep = psA.tile([128, 512], F32, tag="ps512",
                                       name=f"{kind}rep{mt}{i}")
                        nc.tensor.matmul(rep[:, 0:cw], f2[:], rnorm[:, sl],
                                         start=True, stop=True)
                        nc.vector.tensor_tensor(dst[:, sl],
                                                qkT[:, sl].bitcast(F32),
                                                rep[:, 0:cw], ALU.mult)

                def k_proj(mt):
                    pss = [psA.tile([128, 512], F32, tag="ps512",
                                    name=f"k{mt}{nt}") for nt in range(3)]
                    for ct in range(8):
                        for nt in range(3):
                            nc.tensor.matmul(
                                pss[nt][:, 0:384],
                                wk[:, ct, mt * 128:(mt + 1) * 128],
                                xTk[:, ct, nt * 384:(nt + 1) * 384],
                                start=(ct == 0), stop=(ct == 7))
                    chain(mt, pss, 3, 384, ek2[:], "k")

                def q_proj(mt):
                    pss = [psA.tile([128, 512], F32, tag="ps512",
                                    name=f"q{mt}{nt}") for nt in range(4)]
                    for ct in range(8):
                        for nt in range(4):
                            nc.tensor.matmul(
                                pss[nt][:],
                                wq[:, ct, mt * 128:(mt + 1) * 128],
                                xT[:, ct, nt * 512:(nt + 1) * 512],
                                start=(ct == 0), stop=(ct == 7))
                    chain(mt, pss, 4, 512, eqs[:, 2 * mt:2 * mt + 2], "q")

                def v_proj(tt):
                    vps = psA.tile([128, 512], F32, tag="ps512", name=f"v{tt}")
                    for ct in range(8):
                        nc.tensor.matmul(vps[:, 0:256],
                                         xTk[:, ct, tt * 128:(tt + 1) * 128],
                                         wv[:, ct, :], start=(ct == 0), stop=(ct == 7))
                    nc.scalar.activation(
                        v_sb[:, tt].rearrange("p (h d) -> p h d", h=4)[:, :, 0:64],
                        vps[:, 0:256].rearrange("p (h d) -> p h d", h=4), AF.Copy)

                # head-0 inputs first (k0, q0), then v, then the second head
                # pair; chains are emitted after each projection pair so the
                # PE stream is never blocked by a DVE/ACT round trip.
                k_proj(0)
                q_proj(0)
                for tt in range(ktn):
                    v_proj(tt)
                k_proj(1)
                q_proj(1)

            # =================== PHASE B: attention =========================
            with (
                tc.tile_pool(name="aw", bufs=1) as aw_pool,
                tc.tile_pool(name="p0", bufs=4) as p0_pool,
                tc.tile_pool(name="pp", bufs=4) as p_pool,
                tc.tile_pool(name="st", bufs=3) as st_pool,
                tc.tile_pool(name="psSC", bufs=2, space="PSUM") as psSC,
                tc.tile_pool(name="psOA", bufs=2, space="PSUM") as psOA,
            ):
                projw = aw_pool.tile([128, 8, C], BF16)
                a_nm = aw_pool.tile([128, 8, 512], BF16)

                for h in range(HPC):
                    mtq = h // 2
                    off = 64 * (h % 2)
                    oa = {}
                    sc_t = {}
                    p_t = {}
                    for qc in range(2):
                        oa[qc] = psOA.tile([65, 1024], F32, tag="oa",
                                           name=f"oa{h}{qc}")

                    def sc_mm(qc, kt):
                        sc = psSC.tile([128, 1024], F32, tag="sc",
                                       name=f"sc{h}{qc}{kt}")
                        sc_t[qc, kt] = sc
                        for j in range(2):
                            q0 = qc * 1024 + j * 512
                            nc.tensor.matmul(
                                sc[:, j * 512:(j + 1) * 512],
                                kn_sb[mtq][off:off + 64, kt * 128:(kt + 1) * 128],
                                qn_sb[mtq][off:off + 64, q0:q0 + 512],
                                start=True, stop=True)

                    def pv_mm(qc, kt):
                        for j in range(2):
                            nc.tensor.matmul(
                                oa[qc][:, j * 512:(j + 1) * 512],
                                v_sb[:, kt, h * 65:h * 65 + 65],
                                p_t[qc, kt][:, j * 512:(j + 1) * 512],
                                start=(kt == 0), stop=(kt == ktn - 1))

                    sc_mm(0, 0)
                    sc_mm(1, 0)
                    for kt in range(ktn):
                        for qc in range(2):
                            p0 = p0_pool.tile([128, 1024], BF16, tag="p0",
                                              name=f"p0_{h}{qc}{kt}")
                            nc.scalar.activation(p0[:], sc_t[qc, kt][:], AF.Exp)
                            p = p_pool.tile([128, 1024], BF16, tag="p",
                                            name=f"p{h}{qc}{kt}")
                            p_t[qc, kt] = p
                            nc.vector.tensor_tensor(
                                p[:], p0[:],
                                ea_t[(h, kt)][:, qc * 1024:(qc + 1) * 1024],
                                ALU.mult)
                        for qc in range(2):
                            if kt + 1 < ktn:
                                sc_mm(qc, kt + 1)
                            pv_mm(qc, kt)
                        if h + 1 < HPC:
                            ea_dma(h + 1, kt)
                        if h == 1 and kt < 8:
                            nc.sync.dma_start(projw[:, kt, :],
                                              projw_e.ap()[kt * 128:(kt + 1) * 128, :])

                    for qc in range(2):
                        # denominator: copy ones-row, reciprocal, broadcast via
                        # a DRAM bounce (stride-0 partition read), divide.
                        dd = (h % 2) * 2 + qc
                        dens = st_pool.tile([1, 1024], F32, tag="dens",
                                            name=f"dens{h}{qc}")
                        nc.vector.tensor_copy(dens[:], oa[qc][64:65, :])
                        num0 = st_pool.tile([64, 1024], BF16, tag="num0",
                                            name=f"num0{h}{qc}")
                        nc.vector.tensor_copy(num0[:], oa[qc][0:64, :])
                        rcp = st_pool.tile([1, 1024], F32, tag="rcp",
                                           name=f"rcp{h}{qc}")
                        nc.vector.reciprocal_approx_fast(rcp[:], dens[:])
                        nc.sync.dma_start(drcp[dd][:, :], rcp[:])
                        repb = st_pool.tile([64, 1024], F32, tag="repb",
                                            name=f"repb{h}{qc}")
                        nc.sync.dma_start(
                            repb[:], drcp[dd][:, :].partition_broadcast(64))
                        stg = st_pool.tile([64, 1024], BF16, tag="stg",
                                           name=f"stg{h}{qc}")
                        nc.vector.tensor_tensor(stg[:], num0[:],
                                                repb[:], ALU.mult)
                        nc.gpsimd.dma_start(
                            a2a_in[h // 2].rearrange(
                                "(q i) (hp p) t -> q hp p i t", q=2, hp=2)[qc, h % 2],
                            stg[:].rearrange("p (i t) -> p i t", i=4))

                    if h % 2 == 1:
                        par = h // 2
                        nc.gpsimd.collective_compute(
                            "AllToAll", ALU.bypass,
                            replica_groups=[list(range(N_CORES))],
                            ins=[a2a_in[par].opt()],
                            outs=[a2a_out[par].opt()],
                        )
                        dst = a_nm[:].rearrange("p j (b t) -> p j b t", b=2)
                        for bb in range(2):
                            nc.gpsimd.dma_start(
                                dst[:, par::2, bb, :],
                                a2a_out[par].rearrange(
                                    "(b g) p t -> b p g t", b=2)[bb])

            # =================== PHASE D: output projection =================
            with (
                tc.tile_pool(name="od", bufs=1) as od_pool,
                tc.tile_pool(name="psDO", bufs=1, space="PSUM") as psDO,
            ):
                ops = {}
                for tt in range(4):
                    for co in range(2):
                        ops[tt, co] = psDO.tile([128, 512], F32, tag=f"do{tt}{co}",
                                                name=f"do{tt}{co}")
                # even channel tiles overlap the last collective's latency
                for j in (0, 2, 4, 6, 1, 3, 5, 7):
                    for tt in range(4):
                        for co in range(2):
                            nc.tensor.matmul(ops[tt, co][:],
                                             a_nm[:, j, tt * 128:(tt + 1) * 128],
                                             projw[:, j, co * 512:(co + 1) * 512],
                                             start=(j == 0), stop=False)
                o_sb = od_pool.tile([128, 4, C], F32)
                for tt in range(4):
                    for co in range(2):
                        nc.tensor.matmul(ops[tt, co][:], onesb[:],
                                         projb[:, co * 512:(co + 1) * 512],
                                         start=False, stop=True)
                        if co == 0:
                            nc.scalar.activation(o_sb[:, tt, co * 512:(co + 1) * 512],
                                                 ops[tt, co][:], AF.Copy)
                        else:
                            nc.vector.tensor_copy(o_sb[:, tt, co * 512:(co + 1) * 512],
                                                  ops[tt, co][:])
                        oeng = (nc.sync, nc.gpsimd)[(2 * tt + co) % 2]
                        oeng.dma_start(
                            out_e.ap()[tt * 128:(tt + 1) * 128,
                                       co * 512:(co + 1) * 512],
                            o_sb[:, tt, co * 512:(co + 1) * 512])

    nc.compile()
    return nc


def _get_nc(nkp):
    if nkp not in _NCS:
        _NCS[nkp] = _build(nkp)
    return _NCS[nkp]


def kernel(x, padding_mask, alibi_bias, qkv_w, proj_w, proj_b, logit_scale):
    x = np.asarray(x, np.float32)
    padding_mask = np.asarray(padding_mask, bool)
    alibi_bias = np.asarray(alibi_bias, np.float32)
    qkv_w = np.asarray(qkv_w, np.float32)
    proj_w = np.asarray(proj_w, np.float32)
    proj_b = np.asarray(proj_b, np.float32)
    lsc = np.minimum(np.asarray(logit_scale, np.float32).reshape(H), LOG_MAX)

    kept = [np.flatnonzero(~padding_mask[b]) for b in range(B)]
    nk_max = max(len(k) for k in kept)
    nkp = max(NKP_DEFAULT, -(-nk_max // 128) * 128)
    nc = _get_nc(nkp)

    f2 = np.zeros((2, 128), np.float32)
    f2[0, 0:64] = 1.0
    f2[1, 64:128] = 1.0
    ek2 = np.zeros((128, 2), np.float32)
    ek2[0:64, 0] = 1.0
    ek2[64:128, 1] = 1.0
    onesb = np.ones((1, 128), np.float32).astype(BF)
    projw = np.ascontiguousarray(proj_w.T).astype(BF)          # [c_in, c_out]
    projb = proj_b.reshape(1, C).astype(BF)

    in_maps = []
    for c in range(N_CORES):
        b = c // 4
        hs = [4 * (c % 4) + i for i in range(4)]
        idx = kept[b]
        idxp = np.concatenate([idx, np.zeros(nkp - len(idx), np.int64)])
        xT = np.ascontiguousarray(x[b].T).astype(BF)
        xTk = np.ascontiguousarray(x[b].T[:, idxp]).astype(BF)
        wq = np.ascontiguousarray(
            np.concatenate([qkv_w[h * D:(h + 1) * D, :] for h in hs], 0).T).astype(BF)
        wk = np.ascontiguousarray(
            np.concatenate([qkv_w[C + h * D:C + (h + 1) * D, :] for h in hs], 0).T).astype(BF)
        wv = np.ascontiguousarray(
            np.concatenate([qkv_w[2 * C + h * D:2 * C + (h + 1) * D, :] for h in hs], 0).T).astype(BF)
        # ealibi: [h, k_kept, q], exp(alibi); padded key rows -> 0
        ea = np.exp(alibi_bias[b][hs][:, :, idxp].transpose(0, 2, 1))
        if len(idx) < nkp:
            ea[:, len(idx):, :] = 0.0
        ea = np.ascontiguousarray(ea).astype(BF)
        eqs = np.zeros((128, 4), np.float32)
        for j in range(4):
            eqs[(j % 2) * 64:(j % 2) * 64 + 64, j] = np.exp(-2.0 * lsc[hs[j]])
        in_maps.append({
            "xT": xT, "xTk": xTk, "wq": wq, "wk": wk, "wv": wv, "ea": ea,
            "eqs": eqs, "ek2": ek2, "f2": f2,
            "onesb": onesb, "projw": projw, "projb": projb,
        })

    res = run_bass_kernel_spmd(nc, in_maps, core_ids=list(range(N_CORES)),
                               trace=TRACE)
    if TRACE:
        kernel.last_exec_time_ns = res.exec_time_ns
        kernel.last_results = res

    out = np.empty((B, N, C), np.float32)
    for c in range(N_CORES):
        out[0, 256 * c:256 * (c + 1), :] = res.results[c]["out"][0:256]
        out[1, 256 * c:256 * (c + 1), :] = res.results[c]["out"][256:512]
    return out


# revision 24
# speedup vs baseline: 1.2275x; 1.0513x over previous
"""Distributed Trainium2 Bass kernel for AltAttention (cosine-sim attention with
alibi bias + key padding mask + out projection).

Sharding (8 cores): core c -> batch b = c//4, heads [4*(c%4) .. 4*(c%4)+3].

Key ideas vs the v0 baseline:
 - Masked keys are compacted away on the host: only the ~1024 unmasked keys
   per batch are projected/scored (padded to NKP=1152 = 9 k-tiles; padding
   keys get ealibi == 0 so they contribute exactly nothing).
 - exp(alibi) is precomputed on the host (bf16).  On device the softmax
   numerator is exp(s) * ealibi: the Scalar engine does nothing but Exp in
   the attention loop, and the alibi application is a bf16 SBUF x SBUF DVE
   multiply (2x packed mode) instead of an f32 PSUM add.
 - The two query halves of each head run interleaved so the PE always has
   independent matmul work while an exp/multiply round-trip is in flight
   (2 score tiles + 2 PV accumulators = exactly 8 PSUM banks).
 - Output ownership is token-chunked: core c produces tokens
   [256c, 256c+256) of BOTH batches, which makes every AllToAll slot index
   batch-independent -> no duplicated sends, no receiver-side selection.
   Division by the softmax denominator happens sender-side via a
   reciprocal + DRAM-bounce partition broadcast (all off the PE/ACT path).
 - TWO AllToAlls (one per head pair) instead of four: collectives have a
   large fixed re-arm cost; the first overlaps the second pair's compute.
 - Final projection accumulates even channel tiles during the last
   collective and odd tiles after it; bias via a ones-row matmul.
"""

import numpy as np
import ml_dtypes

import concourse.bass as bass
import concourse.mybir as mybir
import concourse.tile as tile
from concourse import bacc
from concourse.bass_utils import run_bass_kernel_spmd

BF = ml_dtypes.bfloat16
F32 = mybir.dt.float32
F32R = mybir.dt.float32r
BF16 = mybir.dt.bfloat16
AF = mybir.ActivationFunctionType
ALU = mybir.AluOpType

B, N, C, H = 2, 2048, 1024, 16
D = C // H
LOG_MAX = float(np.log(1.0 / 0.01))
N_CORES = 8
HPC = 4               # heads per core
NKP_DEFAULT = 1152    # padded compacted key count (9 tiles of 128)

TRACE = False
_NCS = {}


def _build(nkp):
    ktn = nkp // 128
    nc = bacc.Bacc("TRN2", target_bir_lowering=False, debug=False,
                   num_devices=N_CORES)

    xT_e = nc.dram_tensor("xT", [C, N], BF16, kind="ExternalInput")
    xTk_e = nc.dram_tensor("xTk", [C, nkp], BF16, kind="ExternalInput")
    wq_e = nc.dram_tensor("wq", [C, 256], BF16, kind="ExternalInput")
    wk_e = nc.dram_tensor("wk", [C, 256], BF16, kind="ExternalInput")
    wv_e = nc.dram_tensor("wv", [C, 256], BF16, kind="ExternalInput")
    ea_e = nc.dram_tensor("ea", [HPC, nkp, N], BF16, kind="ExternalInput")
    eqs_e = nc.dram_tensor("eqs", [128, 4], F32R, kind="ExternalInput")
    ek2_e = nc.dram_tensor("ek2", [128, 2], F32R, kind="ExternalInput")
    f2_e = nc.dram_tensor("f2", [2, 128], F32R, kind="ExternalInput")
    onesb_e = nc.dram_tensor("onesb", [1, 128], BF16, kind="ExternalInput")
    projw_e = nc.dram_tensor("projw", [C, C], BF16, kind="ExternalInput")
    projb_e = nc.dram_tensor("projb", [1, C], BF16, kind="ExternalInput")
    out_e = nc.dram_tensor("out", [512, C], F32, kind="ExternalOutput")

    with tile.TileContext(nc) as tc:
        with (
            tc.tile_pool(name="consts", bufs=1) as cpool,
            tc.tile_pool(name="qn", bufs=1) as qn_pool,
            tc.tile_pool(name="kn", bufs=1) as kn_pool,
            tc.tile_pool(name="vsb", bufs=1) as v_pool,
            tc.tile_pool(name="ea", bufs=2 * ktn - 1) as ea_pool,
            tc.tile_pool(name="dram", bufs=1, space="DRAM") as dram,
        ):
            eqs = cpool.tile([128, 4], F32R)
            ek2 = cpool.tile([128, 2], F32R)
            f2 = cpool.tile([2, 128], F32R)
            onesb = cpool.tile([1, 128], BF16)
            projb = cpool.tile([1, C], BF16)

            qn_sb = [qn_pool.tile([128, N], BF16, name=f"qn{m}") for m in range(2)]
            kn_sb = [kn_pool.tile([128, nkp], BF16, name=f"kn{m}") for m in range(2)]
            v_sb = v_pool.tile([128, ktn, HPC * 65], BF16)

            ea_t = {}

            def ea_dma(h, kt):
                t = ea_pool.tile([128, N], BF16, tag="ea", name=f"ea{h}_{kt}")
                ea_t[(h, kt)] = t
                eng = (nc.sync, nc.gpsimd)[kt % 2]
                eng.dma_start(t[:], ea_e.ap()[h, kt * 128:(kt + 1) * 128, :])

            # one collective per head pair: slot s rows 0:64 = even head,
            # 64:128 = odd head of the pair; [8, 128, 256] bf16
            a2a_in = [dram.tile([8, 128, 256], BF16, name=f"a2ai{p}") for p in range(2)]
            a2a_out = [dram.tile([8, 128, 256], BF16, name=f"a2ao{p}") for p in range(2)]
            drcp = [dram.tile([1, 1024], F32, name=f"drcp{i}") for i in range(4)]

            # =================== PHASE A: q/k/v projection + normalize ======
            with (
                tc.tile_pool(name="xw", bufs=1) as xw_pool,
                tc.tile_pool(name="qkT", bufs=2) as qkT_pool,
                tc.tile_pool(name="sq", bufs=1) as sq_pool,
                tc.tile_pool(name="sm", bufs=2) as sm_pool,
                tc.tile_pool(name="psA", bufs=6, space="PSUM") as psA,
                tc.tile_pool(name="psS", bufs=1, space="PSUM") as psS,
            ):
                warm = sm_pool.tile([1, 16], F32, tag="rr", name="warm")
                nc.vector.memset(warm[:], 1.0)
                nc.scalar.activation(warm[:], warm[:], AF.Exp)
                nc.scalar.activation(warm[:], warm[:], AF.Sqrt)
                for t, e in ((eqs, eqs_e), (ek2, ek2_e), (f2, f2_e),
                             (onesb, onesb_e), (projb, projb_e)):
                    nc.gpsimd.dma_start(t[:], e.ap())

                wq = xw_pool.tile([128, 8, 256], BF16)
                wk = xw_pool.tile([128, 8, 256], BF16)
                wv = xw_pool.tile([128, 8, 256], BF16)
                xT = xw_pool.tile([128, 8, N], BF16)
                xTk = xw_pool.tile([128, 8, nkp], BF16)
                # split the phase-A loads over all three DMA queues:
                # sync: wk + xTk (k-proj critical path); scalar: xT[0:6];
                # gpsimd: consts + wq + wv + xT[6:8]; then ealibi h0 3-way.
                for ct in range(8):
                    nc.sync.dma_start(wk[:, ct, :], wk_e.ap()[ct * 128:(ct + 1) * 128, :])
                for ct in range(8):
                    nc.gpsimd.dma_start(wq[:, ct, :], wq_e.ap()[ct * 128:(ct + 1) * 128, :])
                for ct in range(8):
                    nc.sync.dma_start(xTk[:, ct, :], xTk_e.ap()[ct * 128:(ct + 1) * 128, :])
                for ct in range(3):
                    nc.scalar.dma_start(xT[:, ct, :], xT_e.ap()[ct * 128:(ct + 1) * 128, :])
                for ct in range(3, 5):
                    nc.sync.dma_start(xT[:, ct, :], xT_e.ap()[ct * 128:(ct + 1) * 128, :])
                for ct in range(5, 8):
                    nc.gpsimd.dma_start(xT[:, ct, :], xT_e.ap()[ct * 128:(ct + 1) * 128, :])
                for ct in range(8):
                    nc.gpsimd.dma_start(wv[:, ct, :], wv_e.ap()[ct * 128:(ct + 1) * 128, :])
                for kt in range(ktn):
                    ea_dma(0, kt)

                for h in range(HPC):
                    nc.vector.memset(v_sb[:, :, h * 65 + 64], 1.0)

                # ---- normalize chain (emitted after the projection matmuls
                # of the group so the PE stream never blocks on ACT/DVE) ----
                def chain(mt, pss, nchunk, cw, elhs, kind, dve_copy=False):
                    tot = nchunk * cw
                    qkT = qkT_pool.tile([128, tot], F32R, tag="qkT",
                                        name=f"{kind}T{mt}")
                    sq = sq_pool.tile([128, tot], F32R, tag="sq",
                                      name=f"{kind}sq{mt}")
                    for i in range(nchunk):
                        sl = slice(i * cw, (i + 1) * cw)
                        if dve_copy:
                            nc.vector.tensor_copy(qkT[:, sl].bitcast(F32),
                                                  pss[i][:, 0:cw])
                        else:
                            nc.scalar.activation(qkT[:, sl], pss[i][:, 0:cw], AF.Copy)
                        nc.vector.tensor_tensor(sq[:, sl], pss[i][:, 0:cw],
                                                qkT[:, sl].bitcast(F32), ALU.mult)
                    rnorm = sm_pool.tile([2, tot], F32R, tag="rn",
                                         name=f"{kind}rn{mt}")
                    for i in range(nchunk):
                        sl = slice(i * cw, (i + 1) * cw)
                        ssp = psS.tile([2, 1024], F32, tag="ssp",
                                       name=f"{kind}ss{mt}{i}")
                        nc.tensor.matmul(ssp[:, 0:cw], elhs, sq[:, sl],
                                         start=True, stop=True)
                        rr = sm_pool.tile([2, 1024], F32, tag="rr",
                                          name=f"{kind}rr{mt}{i}")
                        nc.vector.reciprocal_approx_fast(rr[:, 0:cw], ssp[:, 0:cw])
                        nc.scalar.activation(rnorm[:, sl], rr[:, 0:cw], AF.Sqrt)
                    dst = qn_sb[mt] if kind == "q" else kn_sb[mt]
                    for i in range(nchunk):
                        sl = slice(i * cw, (i + 1) * cw)
                        rep = psA.tile([128, 512], F32, tag="ps512",
                                       name=f"{kind}rep{mt}{i}")
                        nc.tensor.matmul(rep[:, 0:cw], f2[:], rnorm[:, sl],
                                         start=True, stop=True)
                        nc.vector.tensor_tensor(dst[:, sl],
                                                qkT[:, sl].bitcast(F32),
                                                rep[:, 0:cw], ALU.mult)

                def k_proj(mt):
                    pss = [psA.tile([128, 512], F32, tag="ps512",
                                    name=f"k{mt}{nt}") for nt in range(3)]
                    for ct in range(8):
                        for nt in range(3):
                            nc.tensor.matmul(
                                pss[nt][:, 0:384],
                                wk[:, ct, mt * 128:(mt + 1) * 128],
                                xTk[:, ct, nt * 384:(nt + 1) * 384],
                                start=(ct == 0), stop=(ct == 7))
                    return pss

                def q_proj(mt):
                    pss = [psA.tile([128, 512], F32, tag="ps512",
                                    name=f"q{mt}{nt}") for nt in range(4)]
                    for ct in range(8):
                        for nt in range(4):
                            nc.tensor.matmul(
                                pss[nt][:],
                                wq[:, ct, mt * 128:(mt + 1) * 128],
                                xT[:, ct, nt * 512:(nt + 1) * 512],
                                start=(ct == 0), stop=(ct == 7))
                    return pss

                def v_proj(tt):
                    vps = psA.tile([128, 512], F32, tag="ps512", name=f"v{tt}")
                    for ct in range(8):
                        nc.tensor.matmul(vps[:, 0:256],
                                         xTk[:, ct, tt * 128:(tt + 1) * 128],
                                         wv[:, ct, :], start=(ct == 0), stop=(ct == 7))
                    nc.scalar.activation(
                        v_sb[:, tt].rearrange("p (h d) -> p h d", h=4)[:, :, 0:64],
                        vps[:, 0:256].rearrange("p (h d) -> p h d", h=4), AF.Copy)

                # head-0 inputs first (k0, q0), then v, then the second head
                # pair; each chain is emitted after the NEXT projection block
                # so its ACT/DVE round trips overlap PE matmuls.
                kp0 = k_proj(0)
                qp0 = q_proj(0)
                chain(0, kp0, 3, 384, ek2[:], "k")
                for tt in range(4):
                    v_proj(tt)
                chain(0, qp0, 4, 512, eqs[:, 0:2], "q")
                for tt in range(4, ktn):
                    v_proj(tt)
                kp1 = k_proj(1)
                qp1 = q_proj(1)
                chain(1, kp1, 3, 384, ek2[:], "k")
                chain(1, qp1, 4, 512, eqs[:, 2:4], "q")

            # =================== PHASE B: attention =========================
            with (
                tc.tile_pool(name="aw", bufs=1) as aw_pool,
                tc.tile_pool(name="p0", bufs=4) as p0_pool,
                tc.tile_pool(name="pp", bufs=4) as p_pool,
                tc.tile_pool(name="st", bufs=3) as st_pool,
                tc.tile_pool(name="psSC", bufs=2, space="PSUM") as psSC,
                tc.tile_pool(name="psOA", bufs=2, space="PSUM") as psOA,
            ):
                projw = aw_pool.tile([128, 8, C], BF16)
                a_nm = aw_pool.tile([128, 8, 512], BF16)

                for h in range(HPC):
                    mtq = h // 2
                    off = 64 * (h % 2)
                    oa = {}
                    sc_t = {}
                    p_t = {}
                    for qc in range(2):
                        oa[qc] = psOA.tile([65, 1024], F32, tag="oa",
                                           name=f"oa{h}{qc}")

                    def sc_mm(qc, kt):
                        sc = psSC.tile([128, 1024], F32, tag="sc",
                                       name=f"sc{h}{qc}{kt}")
                        sc_t[qc, kt] = sc
                        for j in range(2):
                            q0 = qc * 1024 + j * 512
                            nc.tensor.matmul(
                                sc[:, j * 512:(j + 1) * 512],
                                kn_sb[mtq][off:off + 64, kt * 128:(kt + 1) * 128],
                                qn_sb[mtq][off:off + 64, q0:q0 + 512],
                                start=True, stop=True)

                    def pv_mm(qc, kt):
                        for j in range(2):
                            nc.tensor.matmul(
                                oa[qc][:, j * 512:(j + 1) * 512],
                                v_sb[:, kt, h * 65:h * 65 + 65],
                                p_t[qc, kt][:, j * 512:(j + 1) * 512],
                                start=(kt == 0), stop=(kt == ktn - 1))

                    sc_mm(0, 0)
                    sc_mm(1, 0)
                    for kt in range(ktn):
                        for qc in range(2):
                            p0 = p0_pool.tile([128, 1024], BF16, tag="p0",
                                              name=f"p0_{h}{qc}{kt}")
                            nc.scalar.activation(p0[:], sc_t[qc, kt][:], AF.Exp)
                            p = p_pool.tile([128, 1024], BF16, tag="p",
                                            name=f"p{h}{qc}{kt}")
                            p_t[qc, kt] = p
                            nc.vector.tensor_tensor(
                                p[:], p0[:],
                                ea_t[(h, kt)][:, qc * 1024:(qc + 1) * 1024],
                                ALU.mult)
                        for qc in range(2):
                            if kt + 1 < ktn:
                                sc_mm(qc, kt + 1)
                            pv_mm(qc, kt)
                        if h + 1 < HPC:
                            ea_dma(h + 1, kt)
                        if h == 1 and kt < 8:
                            nc.sync.dma_start(projw[:, kt, :],
                                              projw_e.ap()[kt * 128:(kt + 1) * 128, :])

                    for qc in range(2):
                        # denominator: copy ones-row, reciprocal, broadcast via
                        # a DRAM bounce (stride-0 partition read), divide.
                        dd = (h % 2) * 2 + qc
                        dens = st_pool.tile([1, 1024], F32, tag="dens",
                                            name=f"dens{h}{qc}")
                        nc.vector.tensor_copy(dens[:], oa[qc][64:65, :])
                        num0 = st_pool.tile([64, 1024], BF16, tag="num0",
                                            name=f"num0{h}{qc}")
                        nc.vector.tensor_copy(num0[:], oa[qc][0:64, :])
                        rcp = st_pool.tile([1, 1024], F32, tag="rcp",
                                           name=f"rcp{h}{qc}")
                        nc.vector.reciprocal_approx_fast(rcp[:], dens[:])
                        nc.sync.dma_start(drcp[dd][:, :], rcp[:])
                        repb = st_pool.tile([64, 1024], F32, tag="repb",
                                            name=f"repb{h}{qc}")
                        nc.sync.dma_start(
                            repb[:], drcp[dd][:, :].partition_broadcast(64))
                        stg = st_pool.tile([64, 1024], BF16, tag="stg",
                                           name=f"stg{h}{qc}")
                        nc.vector.tensor_tensor(stg[:], num0[:],
                                                repb[:], ALU.mult)
                        nc.gpsimd.dma_start(
                            a2a_in[h // 2].rearrange(
                                "(q i) (hp p) t -> q hp p i t", q=2, hp=2)[qc, h % 2],
                            stg[:].rearrange("p (i t) -> p i t", i=4))

                    if h % 2 == 1:
                        par = h // 2
                        nc.gpsimd.collective_compute(
                            "AllToAll", ALU.bypass,
                            replica_groups=[list(range(N_CORES))],
                            ins=[a2a_in[par].opt()],
                            outs=[a2a_out[par].opt()],
                        )
                        dst = a_nm[:].rearrange("p j (b t) -> p j b t", b=2)
                        for bb in range(2):
                            nc.gpsimd.dma_start(
                                dst[:, par::2, bb, :],
                                a2a_out[par].rearrange(
                                    "(b g) p t -> b p g t", b=2)[bb])

            # =================== PHASE D: output projection =================
            with (
                tc.tile_pool(name="od", bufs=1) as od_pool,
                tc.tile_pool(name="psDO", bufs=1, space="PSUM") as psDO,
            ):
                ops = {}
                for tt in range(4):
                    for co in range(2):
                        ops[tt, co] = psDO.tile([128, 512], F32, tag=f"do{tt}{co}",
                                                name=f"do{tt}{co}")
                # even channel tiles overlap the last collective's latency
                for j in (0, 2, 4, 6, 1, 3, 5, 7):
                    for tt in range(4):
                        for co in range(2):
                            nc.tensor.matmul(ops[tt, co][:],
                                             a_nm[:, j, tt * 128:(tt + 1) * 128],
                                             projw[:, j, co * 512:(co + 1) * 512],
                                             start=(j == 0), stop=False)
                o_sb = od_pool.tile([128, 4, C], F32)
                for tt in range(4):
                    for co in range(2):
                        nc.tensor.matmul(ops[tt, co][:], onesb[:],
                                         projb[:, co * 512:(co + 1) * 512],
                                         start=False, stop=True)
                        if co == 0:
                            nc.scalar.activation(o_sb[:, tt, co * 512:(co + 1) * 512],
                                                 ops[tt, co][:], AF.Copy)
                        else:
                            nc.vector.tensor_copy(o_sb[:, tt, co * 512:(co + 1) * 512],
                                                  ops[tt, co][:])
                        oeng = (nc.sync, nc.gpsimd)[(2 * tt + co) % 2]
                        oeng.dma_start(
                            out_e.ap()[tt * 128:(tt + 1) * 128,
                                       co * 512:(co + 1) * 512],
                            o_sb[:, tt, co * 512:(co + 1) * 512])

    nc.compile()
    return nc


def _get_nc(nkp):
    if nkp not in _NCS:
        _NCS[nkp] = _build(nkp)
    return _NCS[nkp]


def kernel(x, padding_mask, alibi_bias, qkv_w, proj_w, proj_b, logit_scale):
    x = np.asarray(x, np.float32)
    padding_mask = np.asarray(padding_mask, bool)
    alibi_bias = np.asarray(alibi_bias, np.float32)
    qkv_w = np.asarray(qkv_w, np.float32)
    proj_w = np.asarray(proj_w, np.float32)
    proj_b = np.asarray(proj_b, np.float32)
    lsc = np.minimum(np.asarray(logit_scale, np.float32).reshape(H), LOG_MAX)

    kept = [np.flatnonzero(~padding_mask[b]) for b in range(B)]
    nk_max = max(len(k) for k in kept)
    nkp = max(NKP_DEFAULT, -(-nk_max // 128) * 128)
    nc = _get_nc(nkp)

    f2 = np.zeros((2, 128), np.float32)
    f2[0, 0:64] = 1.0
    f2[1, 64:128] = 1.0
    ek2 = np.zeros((128, 2), np.float32)
    ek2[0:64, 0] = 1.0
    ek2[64:128, 1] = 1.0
    onesb = np.ones((1, 128), np.float32).astype(BF)
    projw = np.ascontiguousarray(proj_w.T).astype(BF)          # [c_in, c_out]
    projb = proj_b.reshape(1, C).astype(BF)

    in_maps = []
    for c in range(N_CORES):
        b = c // 4
        hs = [4 * (c % 4) + i for i in range(4)]
        idx = kept[b]
        idxp = np.concatenate([idx, np.zeros(nkp - len(idx), np.int64)])
        xT = np.ascontiguousarray(x[b].T).astype(BF)
        xTk = np.ascontiguousarray(x[b].T[:, idxp]).astype(BF)
        wq = np.ascontiguousarray(
            np.concatenate([qkv_w[h * D:(h + 1) * D, :] for h in hs], 0).T).astype(BF)
        wk = np.ascontiguousarray(
            np.concatenate([qkv_w[C + h * D:C + (h + 1) * D, :] for h in hs], 0).T).astype(BF)
        wv = np.ascontiguousarray(
            np.concatenate([qkv_w[2 * C + h * D:2 * C + (h + 1) * D, :] for h in hs], 0).T).astype(BF)
        # ealibi: [h, k_kept, q], exp(alibi); padded key rows -> 0
        ea = np.exp(alibi_bias[b][hs][:, :, idxp].transpose(0, 2, 1))
        if len(idx) < nkp:
            ea[:, len(idx):, :] = 0.0
        ea = np.ascontiguousarray(ea).astype(BF)
        eqs = np.zeros((128, 4), np.float32)
        for j in range(4):
            eqs[(j % 2) * 64:(j % 2) * 64 + 64, j] = np.exp(-2.0 * lsc[hs[j]])
        in_maps.append({
            "xT": xT, "xTk": xTk, "wq": wq, "wk": wk, "wv": wv, "ea": ea,
            "eqs": eqs, "ek2": ek2, "f2": f2,
            "onesb": onesb, "projw": projw, "projb": projb,
        })

    res = run_bass_kernel_spmd(nc, in_maps, core_ids=list(range(N_CORES)),
                               trace=TRACE)
    if TRACE:
        kernel.last_exec_time_ns = res.exec_time_ns
        kernel.last_results = res

    out = np.empty((B, N, C), np.float32)
    for c in range(N_CORES):
        out[0, 256 * c:256 * (c + 1), :] = res.results[c]["out"][0:256]
        out[1, 256 * c:256 * (c + 1), :] = res.results[c]["out"][256:512]
    return out
